# revision 1
# baseline (speedup 1.0000x reference)
"""Trainium2 Bass kernel for CrossBandWindowAttention.

Reference computation (per window item b of B_=2048):
    q = (x @ Wq + bq) * scale      -> (64, 96), 6 heads x 16
    k = cross_x @ Wk + bk          -> (64, 96)
    v = cross_x @ Wv + bv          -> (64, 384), 6 heads x 64
    L_h = q_h k_h^T + rpb_bias_h (+ mask_w)
    A = softmax(L, axis=-1)
    out = (concat_h A_h v_h) @ Wp + bp

Sharding: data-parallel over b_ across 8 cores (256 windows each).
Weights / bias table replicated; rpi+rpb_table folded on host into a
(128, 384) additive bias tile (rows = two windows' 64 tokens, cols =
head-major (h, m)).

Per-core layout strategy (all matmuls contract over the partition dim):
  - x/cross_x loaded natural (tokens on partitions), transposed on-chip
    with PE transposes to xT/cxT (channels on partitions, tokens free).
  - Q/K projections emit qT/kT with heads padded to 32-partition strips
    (zero-padded weight columns host-side) so per-head QK matmuls get
    32-aligned partition bases.
  - QK: 12 small matmuls per window pair -> logits PSUM tile (128, 384)
    rows = [winA 64 tokens; winB 64 tokens], cols = 64h+m.
  - softmax: skip max-subtraction (logits are O(1)); exp via ScalarE;
    per-head sums + reciprocal + broadcast multiply on VectorE.
  - PE-transpose the (128, 384) prob tile -> attn^T per head; AV as 12
    small matmuls into a (128, 384) "proj lhsT" PSUM tile whose rows are
    the channel chunk and cols are pair tokens.
  - proj: 3 accumulating matmuls against Wp chunks -> (128 tokens, 384).
Matmul operands are bitcast to float32r (full-rate fp32 PE mode).
"""

import os
from contextlib import ExitStack

import numpy as np

import concourse.bass as bass
import concourse.mybir as mybir
import concourse.tile as tile
from concourse import bacc
from concourse.bass_utils import run_bass_kernel_spmd
from concourse.masks import make_identity

F32 = mybir.dt.float32
F32R = mybir.dt.float32r
BF16 = mybir.dt.bfloat16
# Attention-core dtype knobs (projections always fp32r/fp32):
#   KERNEL_CORE=f32      -> whole attention core fp32
#   KERNEL_CORE=bf16     -> q/k heads, probabilities, and v in bf16
#   KERNEL_CORE=av       -> only probabilities+v (AV matmul operands) bf16
#   KERNEL_CORE=qk       -> only q/k heads bf16
_MODE = os.environ.get("KERNEL_CORE", "f16")
# KERNEL_HOST_T=1: inputs are pre-transposed on host to channel-major
# (3, 128, T) per core; skips on-chip PE transposes of x/cross_x.
HOST_T = os.environ.get("KERNEL_HOST_T", "0") == "1"
# Batch softmax/transpose/AV ops over 2 window pairs (256 tokens) to
# amortize per-instruction overheads. Needs the PSUM bank freed by HOST_T
# and a 2-byte probability dtype.
PB = 2 if (HOST_T and _MODE in ("f16", "bf16")) else 1
# KERNEL_ABLATE=noattn: replace the attention core with a pass-through
# (timing attribution experiments only — wrong results).
ABLATE = os.environ.get("KERNEL_ABLATE", "")
F16 = mybir.dt.float16
_DTMAP = {"f32": (F32, F32), "bf16": (BF16, BF16), "qk": (BF16, F32),
          "av": (F32, BF16), "f16": (F16, F16), "f16qk": (F16, F32)}
QK_DT, PR_DT = _DTMAP[_MODE]

DIM = 96
HEADS = 6
HD = 16  # head dim for q/k
VD = 64  # head dim for v
N = 64  # tokens per window
C = 384
NCORES = 8
B_TOTAL = 2048
NW_CORE = B_TOTAL // NCORES  # 256 windows per core
GRP = 8  # windows per group (512 tokens)
TOK_G = GRP * N  # 512


def _r(ap):
    return ap


def _build(nw, use_mask, use_bias):
    """Build the per-core Bass module for `nw` windows."""
    nc = bacc.Bacc("TRN2", target_bir_lowering=False, debug=False)

    if HOST_T:
        d_x = nc.dram_tensor("x", [3, 128, nw * N], F32R, kind="ExternalInput").ap()
        d_cx = nc.dram_tensor("cx", [3, 128, nw * N], F32R, kind="ExternalInput").ap()
    else:
        d_x = nc.dram_tensor("x", [nw, N, C], F32, kind="ExternalInput").ap()
        d_cx = nc.dram_tensor("cx", [nw, N, C], F32, kind="ExternalInput").ap()
    d_wq = nc.dram_tensor("wq", [C, DIM], F32R, kind="ExternalInput").ap()
    d_wk = nc.dram_tensor("wk", [C, DIM], F32R, kind="ExternalInput").ap()
    d_wv = nc.dram_tensor("wv", [C, C], F32R, kind="ExternalInput").ap()
    d_wp = nc.dram_tensor("wp", [C, C], F32R, kind="ExternalInput").ap()
    d_bias2 = nc.dram_tensor("bias2", [128, C], F32, kind="ExternalInput").ap()
    if use_bias:
        d_bq = nc.dram_tensor("bq_c", [DIM, 1], F32, kind="ExternalInput").ap()
        d_bk = nc.dram_tensor("bk_c", [DIM, 1], F32, kind="ExternalInput").ap()
        d_bv2 = nc.dram_tensor("bv2", [128, C], F32, kind="ExternalInput").ap()
        d_bp2 = nc.dram_tensor("bp2", [128, C], F32, kind="ExternalInput").ap()
    if use_mask:
        d_mask2 = nc.dram_tensor(
            "mask2", [nw // 2, 128, C], F32, kind="ExternalInput"
        ).ap()
    d_y = nc.dram_tensor("y", [nw, N, C], F32, kind="ExternalOutput").ap()

    if HOST_T:
        x_flat, cx_flat = d_x, d_cx
    else:
        x_flat = d_x.rearrange("w n c -> (w n) c")
        cx_flat = d_cx.rearrange("w n c -> (w n) c")
    y_flat = d_y.rearrange("w n c -> (w n) c")

    n_grp = nw // GRP

    with tile.TileContext(nc) as tc, ExitStack() as ctx:
        const = ctx.enter_context(tc.tile_pool(name="const", bufs=1))
        p_nat = ctx.enter_context(tc.tile_pool(name="p_nat", bufs=3))
        p_xt = ctx.enter_context(tc.tile_pool(name="p_xt", bufs=2))
        p_qk = ctx.enter_context(tc.tile_pool(name="p_qk", bufs=2))
        p_v = ctx.enter_context(tc.tile_pool(name="p_v", bufs=2))
        p_sm = ctx.enter_context(tc.tile_pool(name="p_sm", bufs=2))
        p_at = ctx.enter_context(tc.tile_pool(name="p_at", bufs=2))
        p_pl = ctx.enter_context(tc.tile_pool(name="p_pl", bufs=2))
        p_out = ctx.enter_context(tc.tile_pool(name="p_out", bufs=3))
        # PSUM pools: total bank budget is 8.
        # qk-proj, v-proj and final-proj PSUM tiles share one rotating pool;
        # the attention-chain pools (lps, pps) are double-buffered so
        # consecutive iterations pipeline.
        ps_qkvf = ctx.enter_context(
            tc.tile_pool(name="ps_qkvf", bufs=(3 if HOST_T else 2), space="PSUM")
        )
        ps_te = ctx.enter_context(tc.tile_pool(name="ps_te", bufs=1, space="PSUM"))
        ps_l = ctx.enter_context(tc.tile_pool(name="ps_l", bufs=2, space="PSUM"))
        ps_t = (
            None
            if HOST_T
            else ctx.enter_context(tc.tile_pool(name="ps_t", bufs=1, space="PSUM"))
        )
        ps_p = ctx.enter_context(tc.tile_pool(name="ps_p", bufs=2, space="PSUM"))
        ps_qk = ps_qkvf
        ps_v = ps_qkvf
        ps_f = ps_qkvf

        # ---- constants in SBUF ----
        ident = const.tile([128, 128], F32, name="ident")
        make_identity(nc, ident[:])

        wq_sb = const.tile([128, 3, DIM], F32R, name="wq_sb")
        wk_sb = const.tile([128, 3, DIM], F32R, name="wk_sb")
        wv_sb = const.tile([128, 3, C], F32R, name="wv_sb")
        wp_sb = const.tile([128, 3, C], F32R, name="wp_sb")
        bias2_sb = const.tile([128, PB, C], F32, name="bias2_sb")
        for Ci in range(3):
            sl = slice(128 * Ci, 128 * Ci + 128)
            nc.sync.dma_start(wq_sb[:, Ci], d_wq[sl, :])
            nc.sync.dma_start(wk_sb[:, Ci], d_wk[sl, :])
            nc.sync.dma_start(wv_sb[:, Ci], d_wv[sl, :])
            nc.sync.dma_start(wp_sb[:, Ci], d_wp[sl, :])
        for j in range(PB):
            nc.sync.dma_start(bias2_sb[:, j], d_bias2[:])
        if not use_mask:
            expb2_sb = const.tile([128, PB, C], PR_DT, name="expb2_sb")
            nc.scalar.activation(
                expb2_sb[:].rearrange("p j c -> p (j c)"),
                bias2_sb[:].rearrange("p j c -> p (j c)"),
                mybir.ActivationFunctionType.Exp,
            )
        identc = ident
        if PR_DT != F32:
            identc = const.tile([128, 128], PR_DT, name="identc")
            make_identity(nc, identc[:])
        if use_bias:
            bq_sb = const.tile([DIM, 1], F32, name="bq_sb")
            bk_sb = const.tile([DIM, 1], F32, name="bk_sb")
            bv2_sb = const.tile([128, C], F32, name="bv2_sb")
            bp2_sb = const.tile([128, C], F32, name="bp2_sb")
            nc.sync.dma_start(bq_sb[:], d_bq[:])
            nc.sync.dma_start(bk_sb[:], d_bk[:])
            nc.sync.dma_start(bv2_sb[:], d_bv2[:])
            nc.sync.dma_start(bp2_sb[:], d_bp2[:])

        def transpose_in(src_flat, tok0, tag):
            """Load 512 tokens; on-chip transpose unless HOST_T."""
            xt = p_xt.tile([128, 3, TOK_G], F32R, tag=f"xt_{tag}", name=f"xt_{tag}")
            if HOST_T:
                nc.sync.dma_start(
                    xt[:],
                    src_flat[:, :, tok0 : tok0 + TOK_G].rearrange("c p t -> p c t"),
                )
                return xt
            nat = p_nat.tile([128, 4, C], F32, tag="nat", name=f"nat_{tag}")
            nc.sync.dma_start(
                nat[:],
                src_flat[tok0 : tok0 + TOK_G, :].rearrange(
                    "(t p) c -> p t c", p=128
                ),
            )
            for t in range(4):
                tp = ps_t.tile([128, C], F32, tag="tps", name=f"tps_{tag}{t}")
                for Ci in range(3):
                    nc.tensor.transpose(
                        tp[:, 128 * Ci : 128 * (Ci + 1)],
                        nat[:, t, 128 * Ci : 128 * (Ci + 1)],
                        ident[:],
                    )
                dst = xt[:, :, 128 * t : 128 * (t + 1)]
                srcv = tp[:].rearrange("p (c f) -> p c f", c=3)
                if t % 2 == 0:
                    nc.vector.tensor_copy(dst, srcv)
                else:
                    nc.scalar.copy(dst, srcv)
            return xt

        for g in range(n_grp):
            tok0 = g * TOK_G
            xt = transpose_in(x_flat, tok0, "x")
            cxt = transpose_in(cx_flat, tok0, "c")

            # ---- Q/K projections + head relocation to partition base 0 ----
            def qk_proj(src_t, w, b, tag):
                pq = ps_qk.tile([DIM, TOK_G], F32, tag="qkvf", name=f"pq_{tag}")
                for Ci in range(3):
                    nc.tensor.matmul(
                        pq[:],
                        w[:, Ci],
                        src_t[:, Ci],
                        start=(Ci == 0),
                        stop=(Ci == 2),
                    )
                tmp = p_qk.tile([DIM, TOK_G], QK_DT, tag=f"tmp_{tag}", name=f"tmp_{tag}")
                if use_bias:
                    nc.scalar.activation(
                        tmp[:], pq[:], mybir.ActivationFunctionType.Identity, bias=b[:]
                    )
                else:
                    nc.scalar.copy(tmp[:], pq[:])
                th = p_qk.tile([HD, HEADS, TOK_G], QK_DT, tag=f"th_{tag}", name=f"th_{tag}")
                for h in range(HEADS):
                    eng = nc.scalar if h % 2 else nc.sync
                    eng.dma_start(th[:, h], tmp[HD * h : HD * (h + 1), :])
                return th

            qh = qk_proj(xt, wq_sb, bq_sb if use_bias else None, "q")
            kh = qk_proj(cxt, wk_sb, bk_sb if use_bias else None, "k")
            og = p_out.tile([128, 4, C], F32, tag="og", name="og")

            for pp in range(4 // PB):
                W = PB * C
                tok_pp = 128 * PB * pp
                # ---- V projection per pair (natural layout) ----
                va2 = p_v.tile([64, PB, C], PR_DT, tag="va", name="va2")
                vb2 = p_v.tile([64, PB, C], PR_DT, tag="vb", name="vb2")
                for j in range(PB):
                    ptok = tok_pp + 128 * j
                    vps = ps_v.tile([128, C], F32, tag="qkvf", name="vps")
                    for Ci in range(3):
                        nc.tensor.matmul(
                            vps[:],
                            cxt[:, Ci, ptok : ptok + 128],
                            wv_sb[:, Ci],
                            start=(Ci == 0),
                            stop=(Ci == 2),
                        )
                    if use_bias:
                        nc.vector.tensor_tensor(
                            va2[:, j], vps[0:64, :], bv2_sb[0:64, :],
                            op=mybir.AluOpType.add,
                        )
                        nc.vector.tensor_tensor(
                            vb2[:, j], vps[64:128, :], bv2_sb[64:128, :],
                            op=mybir.AluOpType.add,
                        )
                    else:
                        nc.scalar.copy(va2[:, j], vps[0:64, :])
                        nc.scalar.copy(vb2[:, j], vps[64:128, :])

                if ABLATE == "noattn":
                    for j in range(PB):
                        fps = ps_f.tile([128, C], F32, tag="qkvf", name="fps")
                        for Ci in range(3):
                            nc.tensor.matmul(
                                fps[:],
                                xt[:, Ci, tok_pp + 128 * j : tok_pp + 128 * (j + 1)],
                                wp_sb[:, Ci],
                                start=(Ci == 0),
                                stop=(Ci == 2),
                            )
                        p = PB * pp + j
                        nc.vector.tensor_copy(og[:, p, :], fps[:])
                    continue
                # ---- QK logits per pair -> exp into batched SBUF tile ----
                ee_sb = p_sm.tile([128, W], PR_DT, tag="ee", name="ee_sb")
                if use_mask:
                    e_f32 = p_sm.tile([128, W], F32, tag="e", name="e_f32")
                for j in range(PB):
                    ptok = tok_pp + 128 * j
                    lps = ps_l.tile([128, C], F32, tag="lps", name="lps")
                    for h in range(HEADS):
                        for s in range(2):
                            tok = ptok + 64 * s
                            nc.tensor.matmul(
                                lps[64 * s : 64 * s + 64, 64 * h : 64 * h + 64],
                                qh[:, h, tok : tok + 64],
                                kh[:, h, tok : tok + 64],
                                start=True,
                                stop=True,
                                tile_position=(0, 64 * s),
                            )
                    if use_mask:
                        nc.vector.tensor_tensor(
                            e_f32[:, C * j : C * (j + 1)], lps[:],
                            bias2_sb[:, 0], op=mybir.AluOpType.add,
                        )
                    else:
                        nc.scalar.activation(
                            ee_sb[:, C * j : C * (j + 1)], lps[:],
                            mybir.ActivationFunctionType.Exp,
                        )

                # ---- softmax tail (batched over PB pairs) ----
                if use_mask:
                    m_sb = p_sm.tile([128, PB, C], F32, tag="msk", name="m_sb")
                    for j in range(PB):
                        nc.sync.dma_start(m_sb[:, j], d_mask2[g * 4 + PB * pp + j])
                    nc.vector.tensor_tensor(
                        e_f32[:], e_f32[:], m_sb[:].rearrange("p j c -> p (j c)"),
                        op=mybir.AluOpType.add,
                    )
                    nc.scalar.activation(
                        ee_sb[:], e_f32[:], mybir.ActivationFunctionType.Exp
                    )
                    eeb = ee_sb
                else:
                    eeb = p_sm.tile([128, W], PR_DT, tag="eeb", name="eeb")
                    nc.vector.tensor_tensor(
                        eeb[:], ee_sb[:], expb2_sb[:].rearrange("p j c -> p (j c)"),
                        op=mybir.AluOpType.mult,
                    )
                nh = PB * HEADS
                sums = p_sm.tile([128, nh], F32, tag="sums", name="sums")
                nc.vector.reduce_sum(
                    sums[:],
                    eeb[:].rearrange("p (g m) -> p g m", m=N),
                    axis=mybir.AxisListType.X,
                )
                rec = p_sm.tile([128, nh], F32, tag="rec", name="rec")
                nc.vector.reciprocal(rec[:], sums[:])
                een = p_sm.tile([128, W], PR_DT, tag="een", name="een")
                nc.vector.tensor_tensor(
                    een[:].rearrange("p (g m) -> p g m", m=N),
                    eeb[:].rearrange("p (g m) -> p g m", m=N),
                    rec[:].unsqueeze(2).broadcast_to((128, nh, N)),
                    op=mybir.AluOpType.mult,
                )

                # ---- transpose probs -> attn^T, split by head parity ----
                tps = ps_te.tile([128, W], PR_DT, tag="tpse", name="tps_e")
                for b in range(3 * PB):
                    nc.tensor.transpose(
                        tps[:, 128 * b : 128 * (b + 1)],
                        een[:, 128 * b : 128 * (b + 1)],
                        identc[:],
                    )
                at_e = p_at.tile([64, W], PR_DT, tag="at_e", name="at_e")
                at_o = p_at.tile([64, W], PR_DT, tag="at_o", name="at_o")
                nc.scalar.copy(at_e[:], tps[0:64, :])
                nc.scalar.copy(at_o[:], tps[64:128, :])

                # ---- AV: 12 small matmuls per pair -> proj-lhsT SBUF ----
                pl_sb = p_pl.tile([128, W], F32R, tag="pl", name="pl_sb")
                for j in range(PB):
                    pps = ps_p.tile([128, C], F32, tag="pps", name="pps")
                    for Ci in range(3):
                        for h in (2 * Ci, 2 * Ci + 1):
                            at_t = at_e if h % 2 == 0 else at_o
                            for s in range(2):
                                vsb = va2 if s == 0 else vb2
                                nc.tensor.matmul(
                                    pps[64 * (h % 2) : 64 * (h % 2) + 64,
                                        128 * Ci + 64 * s : 128 * Ci + 64 * s + 64],
                                    vsb[:, j, 64 * h : 64 * h + 64],
                                    at_t[:, C * j + 128 * Ci + 64 * s :
                                         C * j + 128 * Ci + 64 * s + 64],
                                    start=True,
                                    stop=True,
                                    tile_position=(0, 64 * (h % 2)),
                                )
                    nc.vector.tensor_copy(pl_sb[:, C * j : C * (j + 1)], pps[:])

                # ---- output projection (per pair) ----
                for j in range(PB):
                    fps = ps_f.tile([128, C], F32, tag="qkvf", name="fps")
                    for Ci in range(3):
                        nc.tensor.matmul(
                            fps[:],
                            pl_sb[:, C * j + 128 * Ci : C * j + 128 * (Ci + 1)],
                            wp_sb[:, Ci],
                            start=(Ci == 0),
                            stop=(Ci == 2),
                        )
                    p = PB * pp + j
                    if use_bias:
                        nc.vector.tensor_tensor(
                            og[:, p, :], fps[:], bp2_sb[:], op=mybir.AluOpType.add
                        )
                    else:
                        nc.vector.tensor_copy(og[:, p, :], fps[:])
            nc.scalar.dma_start(
                y_flat[tok0 : tok0 + TOK_G, :].rearrange("(t p) c -> p t c", p=128),
                og[:],
            )

    nc.compile()
    return nc


def _prep_host(Wq, bq, Wk, bk, Wv, bv, Wp, bp, rpi, rpb_table, mask):
    scale = HD ** (-0.5)
    Wq = np.asarray(Wq, dtype=np.float32) * scale
    bq = np.asarray(bq, dtype=np.float32) * scale
    Wk = np.asarray(Wk, dtype=np.float32)
    bk = np.asarray(bk, dtype=np.float32)

    bq_c = bq.reshape(DIM, 1).copy()
    bk_c = bk.reshape(DIM, 1).copy()

    tbl = np.asarray(rpb_table, dtype=np.float32)
    rp = np.asarray(rpi).astype(np.int64)
    bias_nmh = tbl[rp.reshape(-1)].reshape(N, N, HEADS)  # (n, m, h)
    b_nm = bias_nmh.transpose(0, 2, 1).reshape(N, C)  # (n, (h, m))
    bias2 = np.concatenate([b_nm, b_nm], axis=0).astype(np.float32)  # (128, C)

    bv2 = np.tile(np.asarray(bv, dtype=np.float32)[None, :], (128, 1))
    bp2 = np.tile(np.asarray(bp, dtype=np.float32)[None, :], (128, 1))

    consts = {
        "wq": np.ascontiguousarray(Wq), "wk": np.ascontiguousarray(Wk),
        "wv": np.ascontiguousarray(np.asarray(Wv, dtype=np.float32)),
        "wp": np.ascontiguousarray(np.asarray(Wp, dtype=np.float32)),
        "bias2": bias2,
    }
    use_bias = bool(
        np.any(bq) or np.any(bk) or np.any(np.asarray(bv)) or np.any(np.asarray(bp))
    )
    if use_bias:
        consts.update({"bq_c": bq_c, "bk_c": bk_c, "bv2": bv2, "bp2": bp2})

    mask = np.asarray(mask, dtype=np.float32)
    use_mask = bool(np.any(mask))
    return consts, use_bias, use_mask, mask


def _mask2_for_core(mask, w0, nw):
    """(nw//2, 128, 384): rows = pair tokens, cols tiled over heads."""
    nwin = mask.shape[0]
    out = np.empty((nw // 2, 128, C), dtype=np.float32)
    for p in range(nw // 2):
        wa = (w0 + 2 * p) % nwin
        wb = (w0 + 2 * p + 1) % nwin
        blk = np.concatenate([mask[wa], mask[wb]], axis=0)  # (128, 64)
        out[p] = np.tile(blk, (1, HEADS))
    return out


_CACHE = {}


def prepare(x, cross_x, rpi, mask, Wq, bq, Wk, bk, Wv, bv, Wp, bp, rpb_table):
    """Host prep + module build; returns (nc, in_maps)."""
    x = np.ascontiguousarray(np.asarray(x, dtype=np.float32))
    cross_x = np.ascontiguousarray(np.asarray(cross_x, dtype=np.float32))
    b_ = x.shape[0]
    assert b_ % NCORES == 0
    nw = b_ // NCORES

    consts, use_bias, use_mask, mask_f = _prep_host(
        Wq, bq, Wk, bk, Wv, bv, Wp, bp, rpi, rpb_table, mask
    )

    key = (nw, use_mask, use_bias, _MODE, HOST_T, PB, ABLATE)
    if key not in _CACHE:
        _CACHE[key] = _build(nw, use_mask, use_bias)
    nc = _CACHE[key]

    def shard(a, i):
        s = a[i * nw : (i + 1) * nw]
        if HOST_T:
            return np.ascontiguousarray(
                s.reshape(-1, C).T.reshape(3, 128, nw * N)
            )
        return s

    in_maps = []
    for i in range(NCORES):
        m = dict(consts)
        m["x"] = shard(x, i)
        m["cx"] = shard(cross_x, i)
        if use_mask:
            m["mask2"] = _mask2_for_core(mask_f, i * nw, nw)
        in_maps.append(m)
    return nc, in_maps


def kernel(x, cross_x, rpi, mask, Wq, bq, Wk, bk, Wv, bv, Wp, bp, rpb_table):
    nc, in_maps = prepare(
        x, cross_x, rpi, mask, Wq, bq, Wk, bk, Wv, bv, Wp, bp, rpb_table
    )
    res = run_bass_kernel_spmd(
        nc,
        in_maps,
        core_ids=list(range(NCORES)),
        trace=bool(int(os.environ.get("KERNEL_TRACE", "0"))),
    )
    out = np.concatenate([res.results[i]["y"] for i in range(NCORES)], axis=0)
    kernel.last_exec_time_ns = res.exec_time_ns
    return out


kernel.last_exec_time_ns = None

